# revision 1
# baseline (speedup 1.0000x reference)
"""Cayley soliton propagator on 8 Trainium2 NeuronCores.

Math: the Hamiltonian stencil H (jnp.roll-based) is a circulant matrix along D,
so the whole Cayley step (I + i*dt/2*H)^-1 (I - i*dt/2*H) is one complex
circulant matrix M, computed on the host from ham_w via an FFT of the stencil
symbol.  M's kernel decays to ~1e-8 beyond +-64 taps, so applying M is a
*banded* circulant matmul.  The device kernel does:
  1. nonlinear phase rotation (intensity, mean over D, sin/cos on ACT)
  2. out = rot @ M as fp16 banded matmuls on the PE (PSUM-accumulated)
Data-parallel over B*S rows across the 8 cores; psi is pre-transposed on the
host so the contraction axis D sits on SBUF partitions (no device transposes).
"""

import math

import numpy as np

import concourse.bass as bass
import concourse.bacc as bacc
import concourse.mybir as mybir
from concourse.bass_utils import run_bass_kernel_spmd
from concourse.tile import TileContext

B, S, D = 8, 2048, 1024
N_CORES = 8
ROWS = B * S // N_CORES          # rows (B*S systems) per core = 2048
RC = 256                         # row-chunk size (pipeline unit)
N_RC = ROWS // RC                # 8
N_DC = D // 128                  # 8 d-blocks of 128 partitions
NUM_SCALES, SPARSITY = 3, 5
HALF_DT = 0.05
F32 = mybir.dt.float32
F16 = mybir.dt.float16
AF = mybir.ActivationFunctionType
ALU = mybir.AluOpType

_cache = {}


def _mm_pieces(dc):
    """Banded MM for d-block dc writes psum cols p in [(dc-1)*128, (dc+1)*128)
    (mod 1024; psum col p holds output index k=(64+p) mod 1024).  Split at the
    1024-wrap and the 512 fp32 PSUM bank boundary.
    Returns list of (bank, col_in_bank, j0, width). j indexes the 256-wide rhs."""
    p0 = ((dc - 1) * 128) % 1024
    pieces = []
    j = 0
    while j < 256:
        p = (p0 + j) % 1024
        lim = 256 - j
        lim = min(lim, 1024 - p)          # wrap split
        lim = min(lim, 512 - (p % 512))   # bank split
        pieces.append((p // 512, p % 512, j, lim))
        j += lim
    return pieces


def _build_program(uniform_alpha):
    nc = bacc.Bacc()
    psi_rt = nc.dram_tensor("psi_rt", [D, ROWS], F16, kind="ExternalInput")
    psi_it = nc.dram_tensor("psi_it", [D, ROWS], F16, kind="ExternalInput")
    mband = nc.dram_tensor("mband", [128, 3 * 256], F16, kind="ExternalInput")
    alpha_in = nc.dram_tensor("alpha", [D], F32, kind="ExternalInput")
    out = nc.dram_tensor("out", [ROWS, 2 * D], F32, kind="ExternalOutput")

    with TileContext(nc) as tc:
        with (
            tc.tile_pool(name="const", bufs=1) as constp,
            tc.tile_pool(name="work", bufs=2) as workp,
            tc.tile_pool(name="rot", bufs=2) as rotp,
            tc.tile_pool(name="small", bufs=3) as smallp,
            tc.tile_pool(name="outb", bufs=3) as outbp,
            tc.tile_pool(name="ps", bufs=3, space="PSUM") as psp,
            tc.tile_pool(name="psred", bufs=2, space="PSUM") as psredp,
        ):
            mband_sb = constp.tile([128, 3 * 256], F16)
            nc.sync.dma_start(out=mband_sb, in_=mband[:, :])
            alpha_sb = constp.tile([128, N_DC], F32)
            nc.sync.dma_start(
                out=alpha_sb, in_=alpha_in.rearrange("(dc p) -> p dc", p=128)
            )
            ones_col = constp.tile([128, 1], F16)
            nc.vector.memset(ones_col, 1.0)
            halfpi = constp.tile([128, 1], F32)
            nc.vector.memset(halfpi, math.pi / 2.0)
            zerob = constp.tile([128, 1], F32)
            nc.vector.memset(zerob, 0.0)

            # whole-tensor fp16 loads (host pre-casts), SBUF free = (dc, r)
            # first chunk's rows load first so rc0 compute starts early
            pr16 = constp.tile([128, N_DC * ROWS], F16)
            pi16 = constp.tile([128, N_DC * ROWS], F16)
            row_splits = [(0, RC), (RC, 2 * RC), (2 * RC, ROWS)]
            for a, b in row_splits:
                for dst, src in ((pr16, psi_rt), (pi16, psi_it)):
                    src_ap = src[:, :]
                    dst3 = dst.rearrange("p (dc r) -> p dc r", dc=N_DC)
                    nc.sync.dma_start(
                        out=dst3[:, :, a:b],
                        in_=bass.AP(
                            tensor=src_ap.tensor,
                            offset=src_ap.offset + a,
                            ap=[[ROWS, 128], [128 * ROWS, N_DC], [1, b - a]],
                        ),
                    )

            def chunk_view(tile, r0, rcw):
                # [128, (dc, RC)] strided view of a [128, (dc, ROWS)] tile,
                # restricted to rows [r0, r0+RC); innermost stride stays 1.
                ap = tile[:, :]
                return bass.AP(
                    tensor=ap.tensor,
                    offset=ap.offset + r0,
                    ap=[list(ap.ap[0]), [ROWS, N_DC], [1, rcw]],
                )

            chunks = [(i * RC, (i + 1) * RC) for i in range(N_RC - 1)]
            chunks += [(ROWS - RC, ROWS - RC // 2), (ROWS - RC // 2, ROWS)]

            def phase_stage(rc, r0, r1):
                RCW = r1 - r0
                W = N_DC * RCW
                prc = chunk_view(pr16, r0, RCW)
                pic = chunk_view(pi16, r0, RCW)
                sq_r = workp.tile([128, W], F16, tag="sq_r")
                sq_i = workp.tile([128, W], F16, tag="sq_i")
                nc.scalar.activation(sq_r, prc, AF.Square)
                nc.vector.tensor_mul(sq_i, pic, pic)
                ssum = workp.tile([128, W], F16, tag="ssum")
                nc.vector.tensor_add(ssum, sq_r, sq_i)

                # per-row sum over D via PE ones-reduction: [1, RC] psum
                ps_red = psredp.tile([1, RCW], F32, tag="psred")
                for dc in range(N_DC):
                    nc.tensor.matmul(
                        ps_red,
                        ones_col,
                        ssum[:, dc * RCW : (dc + 1) * RCW],
                        start=(dc == 0),
                        stop=(dc == N_DC - 1),
                    )
                denom = smallp.tile([1, RCW], F32, tag="denom")
                nc.vector.tensor_scalar(
                    denom, ps_red, 1.0 / float(D), 1e-8, op0=ALU.mult, op1=ALU.add
                )
                minv16 = smallp.tile([1, RCW], F16, tag="minv16")
                with nc.allow_low_precision(reason="minv feeds fp16 phase anyway"):
                    nc.vector.reciprocal(minv16, denom)
                minv_bc = smallp.tile([128, RCW], F16, tag="minvbc")
                nc.gpsimd.partition_broadcast(minv_bc[:, :], minv16[:, :])

                # phase = alpha[d] * intensity * minv ; c = cos, s = sin via ACT
                phs = workp.tile([128, W], F16, tag="phs")
                mb_ap = minv_bc[:, 0:RCW]
                minv_rep = bass.AP(
                    tensor=mb_ap.tensor,
                    offset=mb_ap.offset,
                    ap=[list(mb_ap.ap[0]), [0, N_DC], [1, RCW]],
                )
                nc.vector.tensor_mul(phs, ssum, minv_rep)
                cc = rotp.tile([128, W], F16, tag="cc")
                ss = rotp.tile([128, W], F16, tag="ss")
                if uniform_alpha:
                    asc = alpha_sb[:, 0:1]
                    nc.scalar.activation(cc, phs, AF.Sin, bias=halfpi[:, 0:1], scale=asc)
                    nc.scalar.activation(ss, phs, AF.Sin, bias=zerob[:, 0:1], scale=asc)
                else:
                    for dc in range(N_DC):
                        sl = slice(dc * RCW, (dc + 1) * RCW)
                        nc.scalar.activation(
                            cc[:, sl], phs[:, sl], AF.Sin,
                            bias=halfpi[:, 0:1], scale=alpha_sb[:, dc : dc + 1],
                        )
                        nc.scalar.activation(
                            ss[:, sl], phs[:, sl], AF.Sin,
                            bias=zerob[:, 0:1], scale=alpha_sb[:, dc : dc + 1],
                        )
                # rotation: xr = pr*c - pi*s ; xi = pr*s + pi*c   (fp16, DVE 2x)
                t1 = rotp.tile([128, W], F16, tag="ta")
                t2 = rotp.tile([128, W], F16, tag="tb")
                t3 = rotp.tile([128, W], F16, tag="ta", name=f"t3_{rc}")
                t4 = rotp.tile([128, W], F16, tag="tb", name=f"t4_{rc}")
                xr = rotp.tile([128, W], F16, tag="xr")
                xi = rotp.tile([128, W], F16, tag="xi")
                nc.gpsimd.tensor_mul(t1, cc, prc)
                nc.vector.tensor_mul(t2, pic, ss)
                nc.vector.tensor_sub(xr, t1, t2)
                nc.vector.tensor_mul(t3, prc, ss)
                nc.vector.tensor_mul(t4, pic, cc)
                nc.vector.tensor_add(xi, t3, t4)

                return xr, xi

            def mm_stage(rc, r0, r1, xr, xi):
                RCW = r1 - r0
                # banded circulant matmul, row-blocks of 128
                pst = {}
                for rbl in range(RCW // 128):
                    for comp in ("r", "i"):
                        pst[(rbl, comp)] = psp.tile(
                            [128, D], F32, tag="ps", name=f"ps_{comp}_{rc}_{rbl}",
                        )
                for rbl in range(RCW // 128):
                    plan = []  # ((comp, bank), psum_col, width, lhsT_ap, rhs_ap)
                    for dc in range(N_DC):
                        c0 = dc * RCW + rbl * 128
                        for xt, mat, comp in (
                            (xr, 0, "r"), (xr, 1, "i"), (xi, 2, "r"), (xi, 0, "i"),
                        ):
                            lhsT = xt[:, c0 : c0 + 128]
                            for bank, col, j0, wdt in _mm_pieces(dc):
                                rhs = mband_sb[:, mat * 256 + j0 : mat * 256 + j0 + wdt]
                                plan.append(
                                    ((comp, bank), bank * 512 + col, wdt, lhsT, rhs)
                                )
                    first, last = {}, {}
                    for idx, (key, *_rest) in enumerate(plan):
                        first.setdefault(key, idx)
                        last[key] = idx
                    for idx, (key, col, wdt, lhsT, rhs) in enumerate(plan):
                        nc.tensor.matmul(
                            pst[(rbl, key[0])][:, col : col + wdt],
                            lhsT,
                            rhs,
                            start=(first[key] == idx),
                            stop=(last[key] == idx),
                            skip_group_check=True,
                        )

                    # outbuf col q=2p+ci (psum order); the k=(64+p)%1024
                    # rotation is applied by the split DMA store below.
                    outbuf = outbp.tile([128, 2 * D], F32, tag="ob")
                    ob3 = outbuf.rearrange("p (k two) -> p k two", two=2)
                    nc.scalar.copy(ob3[:, :, 0], pst[(rbl, "r")][:, :])
                    nc.scalar.copy(ob3[:, :, 1], pst[(rbl, "i")][:, :])
                    rb = r0 // 128 + rbl
                    orow = out[rb * 128 : (rb + 1) * 128, :]
                    nc.sync.dma_start(out=orow[:, 128:2048], in_=outbuf[:, 0:1920])
                    nc.sync.dma_start(out=orow[:, 0:128], in_=outbuf[:, 1920:2048])

            pending = None
            for rc, (r0, r1) in enumerate(chunks):
                xrxi = phase_stage(rc, r0, r1)
                if pending is not None:
                    mm_stage(*pending)
                pending = (rc, r0, r1, *xrxi)
            mm_stage(*pending)
    return nc


def _host_mband(ham_w):
    """Band tile of the Cayley circulant: entry [p, m*256+j] = M_m[d, k] at
    relative offset k-d = j-64-p (shift-invariant across d-blocks)."""
    k = np.arange(D)
    lam = np.zeros(D, dtype=np.float64)
    w = np.asarray(ham_w, dtype=np.float64)
    for m in range(NUM_SCALES):
        for j in range(SPARSITY):
            off = (2 ** m) * (j + 1)
            lam += w[m, j] * 2.0 * (1.0 - np.cos(2.0 * np.pi * off * k / D))
    g = (1.0 - 1j * HALF_DT * lam) / (1.0 + 1j * HALF_DT * lam)
    ccol = np.fft.ifft(g)
    rel = (np.arange(256)[None, :] - 64 - np.arange(128)[:, None]) % D
    Mr = ccol.real[rel]
    Mi = ccol.imag[rel]
    return np.concatenate([Mr, Mi, -Mi], axis=1).astype(np.float16)


def kernel(psi_r, psi_i, alpha, ham_w):
    psi_r = np.asarray(psi_r, dtype=np.float32)
    psi_i = np.asarray(psi_i, dtype=np.float32)
    alpha = np.asarray(alpha, dtype=np.float32)

    uniform = bool(np.all(alpha == alpha.flat[0]))
    key = ("nc", uniform)
    if key not in _cache:
        nc = _build_program(uniform)
        nc.finalize()
        _cache[key] = nc
    nc = _cache[key]

    mband = _host_mband(ham_w)
    prT = np.ascontiguousarray(psi_r.reshape(B * S, D).T.astype(np.float16))
    piT = np.ascontiguousarray(psi_i.reshape(B * S, D).T.astype(np.float16))

    in_maps = []
    for c in range(N_CORES):
        sl = slice(c * ROWS, (c + 1) * ROWS)
        in_maps.append(
            {
                "psi_rt": np.ascontiguousarray(prT[:, sl]),
                "psi_it": np.ascontiguousarray(piT[:, sl]),
                "mband": mband,
                "alpha": alpha,
            }
        )
    res = run_bass_kernel_spmd(nc, in_maps, core_ids=list(range(N_CORES)))
    _cache["last_run"] = res
    full = np.concatenate([r["out"] for r in res.results], axis=0)
    return full.reshape(B, S, D, 2)



# revision 42
# speedup vs baseline: 1.2152x; 1.2152x over previous
"""Cayley soliton propagator on 8 Trainium2 NeuronCores.

Math: the Hamiltonian stencil H (jnp.roll-based) is a circulant matrix along D,
so the whole Cayley step (I + i*dt/2*H)^-1 (I - i*dt/2*H) is one complex
circulant matrix M, computed on the host from ham_w via an FFT of the stencil
symbol.  M's kernel decays fast, so applying M is a *banded* circulant matmul
whose half-width h is chosen adaptively from the tail energy.

The per-row intensity normalisation is folded into the host: psi rows are
pre-scaled by s_r = sqrt(k_row / k_glob) (k = alpha / (mean|psi|^2 + 1e-8)) so
the device-side phase is sin/cos(k_glob * (pr^2 + pi^2)) with a single scalar
activation scale; the matmul output rows are descaled by 1/s_r on the host.
This is exact (not an approximation) and removes the on-device mean reduction,
reciprocal, broadcast and normalize multiply.

Device pipeline per 256-row chunk (d on partitions, rows on free dim):
  ssum = pr^2 + pi^2 (DVE/ACT), cc/ss = sin(kg*ssum + {pi/2, 0}) (ACT),
  rotation muls/adds (DVE + Pool), banded matmul on PE (psum col == output k),
  psum->SBUF fp16 eviction (ACT/Pool), one contiguous DMA per 128-row block.
Output DRAM layout is [rows, 2, D] fp16; the host interleaves to [..., D, 2]
float32 and applies the 1/s_r row descale.
"""

import math

import numpy as np

import concourse.bass as bass
import concourse.bacc as bacc
import concourse.mybir as mybir
from concourse.bass_utils import run_bass_kernel_spmd
from concourse.tile import TileContext

B, S, D = 8, 2048, 1024
N_CORES = 8
ROWS = B * S // N_CORES          # rows (B*S systems) per core = 2048
RC = 256                         # row-chunk size (pipeline unit)
N_RC = ROWS // RC                # 8
N_DC = D // 128                  # 8 d-blocks of 128 partitions
NUM_SCALES, SPARSITY = 3, 5
HALF_DT = 0.05
F32 = mybir.dt.float32
F16 = mybir.dt.float16
AF = mybir.ActivationFunctionType
ALU = mybir.AluOpType

_cache = {}


def _pick_h(ham_w):
    """Smallest band half-width whose circulant tail energy is < 5e-3."""
    ccol = _cayley_ccol(ham_w)
    mag2 = np.abs(ccol) ** 2
    dist = np.minimum(np.arange(D), D - np.arange(D))
    tot = mag2.sum()
    for h in (12, 16, 24, 32, 48, 64):
        if math.sqrt(mag2[dist > h].sum() / tot) < 5e-3:
            return h
    return 64


def _cayley_ccol(ham_w):
    k = np.arange(D)
    lam = np.zeros(D, dtype=np.float64)
    w = np.asarray(ham_w, dtype=np.float64)
    for m in range(NUM_SCALES):
        for j in range(SPARSITY):
            off = (2 ** m) * (j + 1)
            lam += w[m, j] * 2.0 * (1.0 - np.cos(2.0 * np.pi * off * k / D))
    g = (1.0 - 1j * HALF_DT * lam) / (1.0 + 1j * HALF_DT * lam)
    return np.fft.ifft(g)


def _host_mband(ham_w, h):
    """Band tile [128, 4*Wb]: entry [p, m*Wb + j] = M_m[d, k] at relative
    offset k-d = j-h-p (shift-invariant across d-blocks).  Blocks m: Mr, Mi,
    -Mi, -Mr.  Far taps wrap to negligible ccol values, so no explicit mask."""
    wb = 128 + 2 * h
    ccol = _cayley_ccol(ham_w)
    rel = (np.arange(wb)[None, :] - h - np.arange(128)[:, None]) % D
    Mr = ccol.real[rel]
    Mi = ccol.imag[rel]
    return np.concatenate([Mr, Mi, -Mi, -Mr], axis=1).astype(np.float16)


def _mm_pieces(dc, h):
    """Banded MM for d-block dc writes psum cols k in [dc*128-h, dc*128+128+h)
    (mod 1024); psum col == output index k.  Split at the 1024-wrap and the
    512-float PSUM bank boundary.  Returns (bank, col_in_bank, j0, width)
    where j indexes the Wb-wide rhs."""
    wb = 128 + 2 * h
    k0 = (dc * 128 - h) % D
    pieces = []
    j = 0
    while j < wb:
        k = (k0 + j) % D
        lim = min(wb - j, D - k, 512 - (k % 512))
        pieces.append((k // 512, k % 512, j, lim))
        j += lim
    return pieces


# --- engine assignment knobs (fractions of each pass done on each engine) ---
# Rotation muls t1/t4 are split DVE/Pool by row range; psum evictions are
# split ACT/Pool by row-block index.
T1_ENG = "gpsimd"
T2_ENG = "gpsimd"
T2_POOL_ROWS = 208
T3_ENG = "vector"
T4_ENG = "vector"
SQI_DVE_FRAC = 0.5    # fraction of sq_i rows on DVE (rest ACT)
# eviction engine pairs per row-block: Pool only — Pool has no phase work, so
# evictions (which depend on the mm stage) never gate the phase pipeline.
# The last row-blocks run after all phase work, so they fan out across
# engines to drain the tail in parallel.
EVICT_PAIRS = [("scalar", "scalar"), ("scalar", "vector")]
EVICT_TAIL_PAIRS = [("scalar", "vector")]
EVICT_TAIL_RBL = 6    # this many final row-blocks use the tail pattern


def _build_program(h, uniform_alpha):
    wb = 128 + 2 * h
    nc = bacc.Bacc()
    psi_rt = nc.dram_tensor("psi_rt", [D, ROWS], F16, kind="ExternalInput")
    psi_it = nc.dram_tensor("psi_it", [D, ROWS], F16, kind="ExternalInput")
    mband = nc.dram_tensor("mband", [128, 4 * wb], F16, kind="ExternalInput")
    kg_in = nc.dram_tensor("kg", [128, N_DC], F32, kind="ExternalInput")
    out = nc.dram_tensor("out", [ROWS, 2 * D], F16, kind="ExternalOutput")

    with TileContext(nc) as tc:
        with (
            tc.tile_pool(name="const", bufs=1) as constp,
            tc.tile_pool(name="work", bufs=3) as workp,
            tc.tile_pool(name="rot", bufs=3) as rotp,
            tc.tile_pool(name="outb", bufs=3) as outbp,
            tc.tile_pool(name="ps", bufs=4, space="PSUM") as psp,
        ):
            halfpi = constp.tile([128, 1], F32)
            nc.vector.memset(halfpi, math.pi / 2.0)
            zerob = constp.tile([128, 1], F32)
            nc.vector.memset(zerob, 0.0)
            # warm the ACT function tables (Sin/Square) during input DMA
            warm = constp.tile([128, 1], F16)
            nc.scalar.activation(warm, halfpi[:, 0:1], AF.Square)
            nc.scalar.activation(warm, halfpi[:, 0:1], AF.Sin, bias=zerob[:, 0:1])

            # whole-tensor fp16 loads (host pre-casts + pre-scales), SBUF
            # free = (dc, r); first chunks' rows load first, then consts,
            # then the remaining rows
            pr16 = constp.tile([128, N_DC * ROWS], F16)
            pi16 = constp.tile([128, N_DC * ROWS], F16)
            mband_sb = constp.tile([128, 4 * wb], F16)
            kg_sb = constp.tile([128, N_DC], F32)

            def load_rows(a, b):
                for dst, src in ((pr16, psi_rt), (pi16, psi_it)):
                    src_ap = src[:, :]
                    dst3 = dst.rearrange("p (dc r) -> p dc r", dc=N_DC)
                    nc.sync.dma_start(
                        out=dst3[:, :, a:b],
                        in_=bass.AP(
                            tensor=src_ap.tensor,
                            offset=src_ap.offset + a,
                            ap=[[ROWS, 128], [128 * ROWS, N_DC], [1, b - a]],
                        ),
                    )

            load_rows(0, 128)
            nc.sync.dma_start(out=kg_sb, in_=kg_in[:, :])
            load_rows(128, 2 * RC)
            nc.sync.dma_start(out=mband_sb, in_=mband[:, :])
            load_rows(2 * RC, ROWS)

            def chunk_view(tile, r0, rcw):
                ap = tile[:, :]
                return bass.AP(
                    tensor=ap.tensor,
                    offset=ap.offset + r0,
                    ap=[list(ap.ap[0]), [ROWS, N_DC], [1, rcw]],
                )

            def _e(name):
                return {"gpsimd": nc.gpsimd, "vector": nc.vector}[name]

            def rview(base_ap, r0, rp0, rp1):
                """[128, (dc, rp1-rp0)] view of rows [r0+rp0, r0+rp1)."""
                return bass.AP(
                    tensor=base_ap.tensor,
                    offset=base_ap.offset + r0 + rp0,
                    ap=[list(base_ap.ap[0]), [ROWS, N_DC], [1, rp1 - rp0]],
                )

            def tview(tile, RCW, rp0, rp1):
                t3 = tile.rearrange("p (dc r) -> p dc r", dc=N_DC)
                return t3[:, :, rp0:rp1]

            def squares_stage(rc, r0, r1):
                """sq_r (ACT) + sq_i (ACT/DVE row-split) for chunk rc."""
                RCW = r1 - r0
                W = N_DC * RCW
                prc = chunk_view(pr16, r0, RCW)
                pic = chunk_view(pi16, r0, RCW)
                sq_r = workp.tile([128, W], F16, tag="sq_r", name=f"sqr_{rc}")
                sq_i = workp.tile([128, W], F16, tag="sq_i", name=f"sqi_{rc}")
                nc.vector.tensor_mul(sq_r, prc, prc)
                rp = RCW - int(RCW * SQI_DVE_FRAC)
                if 0 < rp < RCW:
                    nc.scalar.activation(
                        tview(sq_i, RCW, 0, rp), rview(pic, 0, 0, rp), AF.Square
                    )
                    pv = rview(pic, 0, rp, RCW)
                    nc.vector.tensor_mul(tview(sq_i, RCW, rp, RCW), pv, pv)
                elif rp >= RCW:
                    nc.scalar.activation(sq_i, pic, AF.Square)
                else:
                    nc.vector.tensor_mul(sq_i, pic, pic)
                return sq_r, sq_i

            def rot_stage(rc, r0, r1, sq_r, sq_i):
                RCW = r1 - r0
                W = N_DC * RCW
                prc = chunk_view(pr16, r0, RCW)
                pic = chunk_view(pi16, r0, RCW)
                ssum = workp.tile([128, W], F16, tag="ssum", name=f"ssum_{rc}")
                nc.vector.tensor_add(ssum, sq_r, sq_i)

                # cc = cos(kg*ssum), ss = sin(kg*ssum) via ACT Sin
                cc = rotp.tile([128, W], F16, tag="cc")
                ss = rotp.tile([128, W], F16, tag="ss")
                if uniform_alpha:
                    ksc = kg_sb[:, 0:1]
                    nc.scalar.activation(cc, ssum, AF.Sin, bias=halfpi[:, 0:1], scale=ksc)
                    nc.scalar.activation(ss, ssum, AF.Sin, bias=zerob[:, 0:1], scale=ksc)
                else:
                    for dc in range(N_DC):
                        sl = slice(dc * RCW, (dc + 1) * RCW)
                        nc.scalar.activation(
                            cc[:, sl], ssum[:, sl], AF.Sin,
                            bias=halfpi[:, 0:1], scale=kg_sb[:, dc : dc + 1],
                        )
                        nc.scalar.activation(
                            ss[:, sl], ssum[:, sl], AF.Sin,
                            bias=zerob[:, 0:1], scale=kg_sb[:, dc : dc + 1],
                        )
                # rotation streams for the 6-matmul plan:
                #   t1 = pr*cc, t2 = pi*ss, v = pr*ss + pi*cc
                # (xr = t1 - t2 and xi = v are folded into the PE via signed
                # band blocks).  cc-dependent muls first (ready while ss runs).
                t1 = rotp.tile([128, W], F16, tag="t1")
                t2 = rotp.tile([128, W], F16, tag="t2")
                t4 = rotp.tile([128, W], F16, tag="t4", bufs=2)
                t3 = rotp.tile([128, W], F16, tag="t3", bufs=2)
                v = rotp.tile([128, W], F16, tag="v")
                mid = 2 <= rc < len(chunks) - 2
                _e(T1_ENG if mid else "vector").tensor_mul(t1, cc, prc)
                _e(T4_ENG).tensor_mul(t4, pic, cc)
                rp2 = T2_POOL_ROWS if (mid and T2_ENG == "gpsimd") else 0
                rp2 = min(rp2, RCW)
                if 0 < rp2 < RCW:
                    nc.gpsimd.tensor_mul(
                        tview(t2, RCW, 0, rp2), rview(pic, 0, 0, rp2),
                        tview(ss, RCW, 0, rp2),
                    )
                    nc.vector.tensor_mul(
                        tview(t2, RCW, rp2, RCW), rview(pic, 0, rp2, RCW),
                        tview(ss, RCW, rp2, RCW),
                    )
                elif rp2 >= RCW:
                    nc.gpsimd.tensor_mul(t2, pic, ss)
                else:
                    nc.vector.tensor_mul(t2, pic, ss)
                _e(T3_ENG).tensor_mul(t3, prc, ss)
                nc.vector.tensor_add(v, t3, t4)
                return t1, t2, v

            N_RBL = ROWS // 128

            def mm_matmuls(rc, r0, r1, t1, t2, v):
                RCW = r1 - r0
                psts = []
                for rbl in range(RCW // 128):
                    pst = {}
                    for comp in ("r", "i"):
                        pst[comp] = psp.tile(
                            [128, D], F32, tag="ps", name=f"ps_{comp}_{rc}_{rbl}",
                        )
                    plan = []  # ((comp, bank), psum_col, width, lhsT, rhs)
                    # out_r = Mr*t1 - Mr*t2 - Mi*v ; out_i = Mi*t1 - Mi*t2 + Mr*v
                    # band blocks: 0=Mr, 1=Mi, 2=-Mi, 3=-Mr
                    for dc in range(N_DC):
                        c0 = dc * RCW + rbl * 128
                        for xt, mat, comp in (
                            (t1, 0, "r"), (t1, 1, "i"), (t2, 3, "r"),
                            (t2, 2, "i"), (v, 2, "r"), (v, 0, "i"),
                        ):
                            lhsT = xt[:, c0 : c0 + 128]
                            for bank, col, j0, wdt in _mm_pieces(dc, h):
                                rhs = mband_sb[:, mat * (128 + 2 * h) + j0 :
                                               mat * (128 + 2 * h) + j0 + wdt]
                                plan.append(
                                    ((comp, bank), bank * 512 + col, wdt, lhsT, rhs)
                                )
                    first, last = {}, {}
                    for idx, (key, *_rest) in enumerate(plan):
                        first.setdefault(key, idx)
                        last[key] = idx
                    for idx, (key, col, wdt, lhsT, rhs) in enumerate(plan):
                        nc.tensor.matmul(
                            pst[key[0]][:, col : col + wdt],
                            lhsT,
                            rhs,
                            start=(first[key] == idx),
                            stop=(last[key] == idx),
                            skip_group_check=True,
                        )
                    psts.append(pst)
                return psts

            def mm_evict(rc, r0, r1, psts):
                for rbl, pst in enumerate(psts):
                    # evict psum -> SBUF fp16, layout [2, D] per row
                    outbuf = outbp.tile([128, 2 * D], F16, tag="ob")
                    rb = r0 // 128 + rbl
                    if rb >= N_RBL - EVICT_TAIL_RBL:
                        pair = EVICT_TAIL_PAIRS[rb % len(EVICT_TAIL_PAIRS)]
                    else:
                        pair = EVICT_PAIRS[rb % len(EVICT_PAIRS)]
                    for ci, comp in enumerate(("r", "i")):
                        ename = pair[ci]
                        dst = outbuf[:, ci * D : (ci + 1) * D]
                        if ename == "scalar":
                            nc.scalar.copy(dst, pst[comp][:, :])
                        else:
                            nc.vector.tensor_copy(dst, pst[comp][:, :])
                    nc.sync.dma_start(
                        out=out[rb * 128 : (rb + 1) * 128, :], in_=outbuf[:, :]
                    )

            chunks = [(0, 128), (128, 256)]
            chunks += [(i * RC, (i + 1) * RC) for i in range(1, N_RC - 2)]
            chunks += [(r, r + 128) for r in range(ROWS - 2 * RC, ROWS, 128)]
            # software pipeline: squares(c+1) | sins+rot(c) | matmuls(c-1)
            # | evict+dma(c-2) — evictions land well after their matmuls so
            # they never stall the ACT/DVE phase streams
            sq_pend = None
            rot_done = []   # (rc, r0, r1, t1, t2, v) awaiting matmuls
            mm_done = []    # (rc, r0, r1, psts) awaiting evict
            for rc, (r0, r1) in enumerate(chunks):
                sq = squares_stage(rc, r0, r1)
                if sq_pend is not None:
                    t1t2v = rot_stage(*sq_pend)
                    rot_done.append((sq_pend[0], sq_pend[1], sq_pend[2], *t1t2v))
                    if len(mm_done) > 1:
                        mm_evict(*mm_done.pop(0))
                    if len(rot_done) > 1:
                        args = rot_done.pop(0)
                        psts = mm_matmuls(*args)
                        mm_done.append((args[0], args[1], args[2], psts))
                sq_pend = (rc, r0, r1, *sq)
            t1t2v = rot_stage(*sq_pend)
            rot_done.append((sq_pend[0], sq_pend[1], sq_pend[2], *t1t2v))
            for args in rot_done:
                psts = mm_matmuls(*args)
                mm_done.append((args[0], args[1], args[2], psts))
                while len(mm_done) > 1:
                    mm_evict(*mm_done.pop(0))
            while mm_done:
                mm_evict(*mm_done.pop(0))
    return nc


def kernel(psi_r, psi_i, alpha, ham_w):
    psi_r = np.asarray(psi_r, dtype=np.float32)
    psi_i = np.asarray(psi_i, dtype=np.float32)
    alpha = np.asarray(alpha, dtype=np.float32)

    uniform = bool(np.all(alpha == alpha.flat[0]))
    h = _pick_h(ham_w)
    key = ("prog", h, uniform)
    if key not in _cache:
        nc = _build_program(h, uniform)
        nc.finalize()
        _cache[key] = nc
    nc = _cache[key]
    _cache[("nc", uniform)] = nc  # test.py compatibility

    mband = _host_mband(ham_w, h)

    # host-side normalisation fold: k_row = alpha_scale / (mean I + 1e-8)
    pr = psi_r.reshape(B * S, D)
    pi = psi_i.reshape(B * S, D)
    inten_mean = (
        (pr.astype(np.float64) ** 2 + pi.astype(np.float64) ** 2).mean(axis=1)
    )
    k_row = 1.0 / (inten_mean + 1e-8)
    k_glob = float(np.exp(np.mean(np.log(k_row))))
    s_row = np.sqrt(k_row / k_glob)          # pre-scale; exp(log-mean) keeps ~1
    # per-d activation scale alpha[d] * k_glob, laid out [p, dc] (d = dc*128+p)
    kg = np.ascontiguousarray(
        (alpha * k_glob).reshape(N_DC, 128).T.astype(np.float32)
    )

    sc = s_row.astype(np.float32)[:, None]
    prT = np.ascontiguousarray((pr * sc).T.astype(np.float16))
    piT = np.ascontiguousarray((pi * sc).T.astype(np.float16))

    in_maps = []
    for c in range(N_CORES):
        sl = slice(c * ROWS, (c + 1) * ROWS)
        in_maps.append(
            {
                "psi_rt": np.ascontiguousarray(prT[:, sl]),
                "psi_it": np.ascontiguousarray(piT[:, sl]),
                "mband": mband,
                "kg": kg,
            }
        )
    res = run_bass_kernel_spmd(nc, in_maps, core_ids=list(range(N_CORES)))
    _cache["last_run"] = res
    out16 = np.concatenate([r["out"] for r in res.results], axis=0)
    # [rows, 2, D] fp16 -> [rows, D, 2] f32, descale rows by 1/s_row
    full = out16.reshape(B * S, 2, D).astype(np.float32)
    full *= (1.0 / s_row).astype(np.float32)[:, None, None]
    return np.ascontiguousarray(full.transpose(0, 2, 1)).reshape(B, S, D, 2)


# revision 45
# speedup vs baseline: 1.2206x; 1.0045x over previous
"""Cayley soliton propagator on 8 Trainium2 NeuronCores.

Math: the Hamiltonian stencil H (jnp.roll-based) is a circulant matrix along D,
so the whole Cayley step (I + i*dt/2*H)^-1 (I - i*dt/2*H) is one complex
circulant matrix M, computed on the host from ham_w via an FFT of the stencil
symbol.  M's kernel decays fast, so applying M is a *banded* circulant matmul
whose half-width h is chosen adaptively from the tail energy.

The per-row intensity normalisation is folded into the host: psi rows are
pre-scaled by s_r = sqrt(k_row / k_glob) (k = alpha / (mean|psi|^2 + 1e-8)) so
the device-side phase is sin/cos(k_glob * (pr^2 + pi^2)) with a single scalar
activation scale; the matmul output rows are descaled by 1/s_r on the host.
This is exact (not an approximation) and removes the on-device mean reduction,
reciprocal, broadcast and normalize multiply.

Device pipeline per 256-row chunk (d on partitions, rows on free dim):
  ssum = pr^2 + pi^2 (DVE/ACT), cc/ss = sin(kg*ssum + {pi/2, 0}) (ACT),
  rotation muls/adds (DVE + Pool), banded matmul on PE (psum col == output k),
  psum->SBUF fp16 eviction (ACT/Pool), one contiguous DMA per 128-row block.
Output DRAM layout is [rows, 2, D] fp16; the host interleaves to [..., D, 2]
float32 and applies the 1/s_r row descale.
"""

import math

import numpy as np

import concourse.bass as bass
import concourse.bacc as bacc
import concourse.mybir as mybir
from concourse.bass_utils import run_bass_kernel_spmd
from concourse.tile import TileContext

B, S, D = 8, 2048, 1024
N_CORES = 8
ROWS = B * S // N_CORES          # rows (B*S systems) per core = 2048
RC = 256                         # row-chunk size (pipeline unit)
N_RC = ROWS // RC                # 8
N_DC = D // 128                  # 8 d-blocks of 128 partitions
NUM_SCALES, SPARSITY = 3, 5
HALF_DT = 0.05
F32 = mybir.dt.float32
F16 = mybir.dt.float16
AF = mybir.ActivationFunctionType
ALU = mybir.AluOpType

_cache = {}


def _pick_h(ham_w):
    """Smallest band half-width whose circulant tail energy is < 5e-3."""
    ccol = _cayley_ccol(ham_w)
    mag2 = np.abs(ccol) ** 2
    dist = np.minimum(np.arange(D), D - np.arange(D))
    tot = mag2.sum()
    for h in (12, 16, 24, 32, 48, 64):
        if math.sqrt(mag2[dist > h].sum() / tot) < 5e-3:
            return h
    return 64


def _cayley_ccol(ham_w):
    k = np.arange(D)
    lam = np.zeros(D, dtype=np.float64)
    w = np.asarray(ham_w, dtype=np.float64)
    for m in range(NUM_SCALES):
        for j in range(SPARSITY):
            off = (2 ** m) * (j + 1)
            lam += w[m, j] * 2.0 * (1.0 - np.cos(2.0 * np.pi * off * k / D))
    g = (1.0 - 1j * HALF_DT * lam) / (1.0 + 1j * HALF_DT * lam)
    return np.fft.ifft(g)


def _host_mband(ham_w, h):
    """Band tile [128, 4*Wb]: entry [p, m*Wb + j] = M_m[d, k] at relative
    offset k-d = j-h-p (shift-invariant across d-blocks).  Blocks m: Mr, Mi,
    -Mi, -Mr.  Far taps wrap to negligible ccol values, so no explicit mask."""
    wb = 128 + 2 * h
    ccol = _cayley_ccol(ham_w)
    rel = (np.arange(wb)[None, :] - h - np.arange(128)[:, None]) % D
    Mr = ccol.real[rel]
    Mi = ccol.imag[rel]
    return np.concatenate([Mr, Mi, -Mi, -Mr], axis=1).astype(np.float16)


def _mm_pieces(dc, h):
    """Banded MM for d-block dc writes psum cols k in [dc*128-h, dc*128+128+h)
    (mod 1024); psum col == output index k.  Split at the 1024-wrap and the
    512-float PSUM bank boundary.  Returns (bank, col_in_bank, j0, width)
    where j indexes the Wb-wide rhs."""
    wb = 128 + 2 * h
    k0 = (dc * 128 - h) % D
    pieces = []
    j = 0
    while j < wb:
        k = (k0 + j) % D
        lim = min(wb - j, D - k, 512 - (k % 512))
        pieces.append((k // 512, k % 512, j, lim))
        j += lim
    return pieces


# --- engine assignment knobs (fractions of each pass done on each engine) ---
# Rotation muls t1/t4 are split DVE/Pool by row range; psum evictions are
# split ACT/Pool by row-block index.
T1_ENG = "gpsimd"
T2_ENG = "gpsimd"
T2_POOL_ROWS = 128
V_POOL_ROWS = 0
T3_ENG = "vector"
T4_ENG = "vector"
SQI_DVE_FRAC = 0.5    # fraction of sq_i rows on DVE (rest ACT)
# eviction engine pairs per row-block: Pool only — Pool has no phase work, so
# evictions (which depend on the mm stage) never gate the phase pipeline.
# The last row-blocks run after all phase work, so they fan out across
# engines to drain the tail in parallel.
EVICT_PAIRS = [("scalar", "scalar"), ("scalar", "vector")]
EVICT_TAIL_PAIRS = [("scalar", "vector")]
EVICT_TAIL_RBL = 6    # this many final row-blocks use the tail pattern


def _build_program(h, uniform_alpha):
    wb = 128 + 2 * h
    nc = bacc.Bacc()
    psi_rt = nc.dram_tensor("psi_rt", [D, ROWS], F16, kind="ExternalInput")
    psi_it = nc.dram_tensor("psi_it", [D, ROWS], F16, kind="ExternalInput")
    mband = nc.dram_tensor("mband", [128, 4 * wb], F16, kind="ExternalInput")
    kg_in = nc.dram_tensor("kg", [128, N_DC], F32, kind="ExternalInput")
    out = nc.dram_tensor("out", [ROWS, 2 * D], F16, kind="ExternalOutput")

    with TileContext(nc) as tc:
        with (
            tc.tile_pool(name="const", bufs=1) as constp,
            tc.tile_pool(name="work", bufs=4) as workp,
            tc.tile_pool(name="rot", bufs=4) as rotp,
            tc.tile_pool(name="outb", bufs=3) as outbp,
            tc.tile_pool(name="ps", bufs=2, space="PSUM") as psp,
        ):
            halfpi = constp.tile([128, 1], F32)
            nc.vector.memset(halfpi, math.pi / 2.0)
            zerob = constp.tile([128, 1], F32)
            nc.vector.memset(zerob, 0.0)
            # warm the ACT function tables (Sin/Square) during input DMA
            warm = constp.tile([128, 1], F16)
            nc.scalar.activation(warm, halfpi[:, 0:1], AF.Square)
            nc.scalar.activation(warm, halfpi[:, 0:1], AF.Sin, bias=zerob[:, 0:1])

            # whole-tensor fp16 loads (host pre-casts + pre-scales), SBUF
            # free = (dc, r); first chunks' rows load first, then consts,
            # then the remaining rows
            pr16 = constp.tile([128, N_DC * ROWS], F16)
            pi16 = constp.tile([128, N_DC * ROWS], F16)
            mband_sb = constp.tile([128, 4 * wb], F16)
            kg_sb = constp.tile([128, N_DC], F32)

            def load_rows(a, b):
                for dst, src in ((pr16, psi_rt), (pi16, psi_it)):
                    src_ap = src[:, :]
                    dst3 = dst.rearrange("p (dc r) -> p dc r", dc=N_DC)
                    nc.sync.dma_start(
                        out=dst3[:, :, a:b],
                        in_=bass.AP(
                            tensor=src_ap.tensor,
                            offset=src_ap.offset + a,
                            ap=[[ROWS, 128], [128 * ROWS, N_DC], [1, b - a]],
                        ),
                    )

            load_rows(0, 128)
            nc.sync.dma_start(out=kg_sb, in_=kg_in[:, :])
            load_rows(128, 2 * RC)
            nc.sync.dma_start(out=mband_sb, in_=mband[:, :])
            load_rows(2 * RC, ROWS)

            def chunk_view(tile, r0, rcw):
                ap = tile[:, :]
                return bass.AP(
                    tensor=ap.tensor,
                    offset=ap.offset + r0,
                    ap=[list(ap.ap[0]), [ROWS, N_DC], [1, rcw]],
                )

            def _e(name):
                return {"gpsimd": nc.gpsimd, "vector": nc.vector}[name]

            def rview(base_ap, r0, rp0, rp1):
                """[128, (dc, rp1-rp0)] view of rows [r0+rp0, r0+rp1)."""
                return bass.AP(
                    tensor=base_ap.tensor,
                    offset=base_ap.offset + r0 + rp0,
                    ap=[list(base_ap.ap[0]), [ROWS, N_DC], [1, rp1 - rp0]],
                )

            def tview(tile, RCW, rp0, rp1):
                t3 = tile.rearrange("p (dc r) -> p dc r", dc=N_DC)
                return t3[:, :, rp0:rp1]

            def squares_stage(rc, r0, r1):
                """sq_r (ACT) + sq_i (ACT/DVE row-split) for chunk rc."""
                RCW = r1 - r0
                W = N_DC * RCW
                prc = chunk_view(pr16, r0, RCW)
                pic = chunk_view(pi16, r0, RCW)
                sq_r = workp.tile([128, W], F16, tag="sq_r", name=f"sqr_{rc}")
                sq_i = workp.tile([128, W], F16, tag="sq_i", name=f"sqi_{rc}")
                nc.vector.tensor_mul(sq_r, prc, prc)
                rp = RCW - int(RCW * SQI_DVE_FRAC)
                if 0 < rp < RCW:
                    nc.scalar.activation(
                        tview(sq_i, RCW, 0, rp), rview(pic, 0, 0, rp), AF.Square
                    )
                    pv = rview(pic, 0, rp, RCW)
                    nc.vector.tensor_mul(tview(sq_i, RCW, rp, RCW), pv, pv)
                elif rp >= RCW:
                    nc.scalar.activation(sq_i, pic, AF.Square)
                else:
                    nc.vector.tensor_mul(sq_i, pic, pic)
                return sq_r, sq_i

            def rot_stage(rc, r0, r1, sq_r, sq_i):
                RCW = r1 - r0
                W = N_DC * RCW
                prc = chunk_view(pr16, r0, RCW)
                pic = chunk_view(pi16, r0, RCW)
                ssum = workp.tile([128, W], F16, tag="ssum", name=f"ssum_{rc}")
                nc.vector.tensor_add(ssum, sq_r, sq_i)

                # cc = cos(kg*ssum), ss = sin(kg*ssum) via ACT Sin
                cc = rotp.tile([128, W], F16, tag="cc")
                ss = rotp.tile([128, W], F16, tag="ss")
                if uniform_alpha:
                    ksc = kg_sb[:, 0:1]
                    nc.scalar.activation(cc, ssum, AF.Sin, bias=halfpi[:, 0:1], scale=ksc)
                    nc.scalar.activation(ss, ssum, AF.Sin, bias=zerob[:, 0:1], scale=ksc)
                else:
                    for dc in range(N_DC):
                        sl = slice(dc * RCW, (dc + 1) * RCW)
                        nc.scalar.activation(
                            cc[:, sl], ssum[:, sl], AF.Sin,
                            bias=halfpi[:, 0:1], scale=kg_sb[:, dc : dc + 1],
                        )
                        nc.scalar.activation(
                            ss[:, sl], ssum[:, sl], AF.Sin,
                            bias=zerob[:, 0:1], scale=kg_sb[:, dc : dc + 1],
                        )
                # rotation streams for the 6-matmul plan:
                #   t1 = pr*cc, t2 = pi*ss, v = pr*ss + pi*cc
                # (xr = t1 - t2 and xi = v are folded into the PE via signed
                # band blocks).  cc-dependent muls first (ready while ss runs).
                t1 = rotp.tile([128, W], F16, tag="t1")
                t2 = rotp.tile([128, W], F16, tag="t2")
                t4 = rotp.tile([128, W], F16, tag="t4", bufs=2)
                t3 = rotp.tile([128, W], F16, tag="t3", bufs=2)
                v = rotp.tile([128, W], F16, tag="v")
                mid = 2 <= rc < len(chunks) - 2
                _e(T1_ENG if mid else "vector").tensor_mul(t1, cc, prc)
                _e(T4_ENG).tensor_mul(t4, pic, cc)
                rp2 = T2_POOL_ROWS if (mid and T2_ENG == "gpsimd") else 0
                rp2 = min(rp2, RCW)
                if 0 < rp2 < RCW:
                    nc.gpsimd.tensor_mul(
                        tview(t2, RCW, 0, rp2), rview(pic, 0, 0, rp2),
                        tview(ss, RCW, 0, rp2),
                    )
                    nc.vector.tensor_mul(
                        tview(t2, RCW, rp2, RCW), rview(pic, 0, rp2, RCW),
                        tview(ss, RCW, rp2, RCW),
                    )
                elif rp2 >= RCW:
                    nc.gpsimd.tensor_mul(t2, pic, ss)
                else:
                    nc.vector.tensor_mul(t2, pic, ss)
                _e(T3_ENG).tensor_mul(t3, prc, ss)
                rpv = V_POOL_ROWS if mid else 0
                rpv = min(rpv, RCW)
                if 0 < rpv < RCW:
                    nc.gpsimd.tensor_add(
                        tview(v, RCW, 0, rpv), tview(t3, RCW, 0, rpv),
                        tview(t4, RCW, 0, rpv),
                    )
                    nc.vector.tensor_add(
                        tview(v, RCW, rpv, RCW), tview(t3, RCW, rpv, RCW),
                        tview(t4, RCW, rpv, RCW),
                    )
                elif rpv >= RCW:
                    nc.gpsimd.tensor_add(v, t3, t4)
                else:
                    nc.vector.tensor_add(v, t3, t4)
                return t1, t2, v

            N_RBL = ROWS // 128

            def mm_matmuls(rc, r0, r1, t1, t2, v):
                RCW = r1 - r0
                psts = []
                for rbl in range(RCW // 128):
                    pst2 = psp.tile(
                        [128, 2 * D], F32, tag="ps", name=f"ps_{rc}_{rbl}",
                    )
                    pst = {"r": pst2[:, 0:D], "i": pst2[:, D : 2 * D]}
                    plan = []  # ((comp, bank), psum_col, width, lhsT, rhs)
                    # out_r = Mr*t1 - Mr*t2 - Mi*v ; out_i = Mi*t1 - Mi*t2 + Mr*v
                    # band blocks: 0=Mr, 1=Mi, 2=-Mi, 3=-Mr
                    for dc in range(N_DC):
                        c0 = dc * RCW + rbl * 128
                        for xt, mat, comp in (
                            (t1, 0, "r"), (t1, 1, "i"), (t2, 3, "r"),
                            (t2, 2, "i"), (v, 2, "r"), (v, 0, "i"),
                        ):
                            lhsT = xt[:, c0 : c0 + 128]
                            for bank, col, j0, wdt in _mm_pieces(dc, h):
                                rhs = mband_sb[:, mat * (128 + 2 * h) + j0 :
                                               mat * (128 + 2 * h) + j0 + wdt]
                                plan.append(
                                    ((comp, bank), bank * 512 + col, wdt, lhsT, rhs)
                                )
                    first, last = {}, {}
                    for idx, (key, *_rest) in enumerate(plan):
                        first.setdefault(key, idx)
                        last[key] = idx
                    for idx, (key, col, wdt, lhsT, rhs) in enumerate(plan):
                        nc.tensor.matmul(
                            pst[key[0]][:, col : col + wdt],
                            lhsT,
                            rhs,
                            start=(first[key] == idx),
                            stop=(last[key] == idx),
                            skip_group_check=True,
                        )
                    psts.append(pst2)
                return psts

            def mm_evict(rc, r0, r1, psts):
                for rbl, pst2 in enumerate(psts):
                    # evict psum -> SBUF fp16: two parallel copies (ACT + DVE)
                    outbuf = outbp.tile([128, 2 * D], F16, tag="ob")
                    rb = r0 // 128 + rbl
                    pair = EVICT_PAIRS[rb % len(EVICT_PAIRS)]
                    for ci, ename in enumerate(pair):
                        lo, hi = ci * D, (ci + 1) * D
                        if ename == "scalar":
                            nc.scalar.copy(outbuf[:, lo:hi], pst2[:, lo:hi])
                        else:
                            nc.vector.tensor_copy(outbuf[:, lo:hi], pst2[:, lo:hi])
                    nc.sync.dma_start(
                        out=out[rb * 128 : (rb + 1) * 128, :], in_=outbuf[:, :]
                    )

            chunks = [(r, r + 128) for r in range(0, ROWS, 128)]
            # software pipeline: squares(c+1) | sins+rot(c) | matmuls(c-1)
            # | evict+dma(c-2) — evictions land well after their matmuls so
            # they never stall the ACT/DVE phase streams
            sq_pend = None
            rot_done = []   # (rc, r0, r1, t1, t2, v) awaiting matmuls
            mm_done = []    # (rc, r0, r1, psts) awaiting evict
            for rc, (r0, r1) in enumerate(chunks):
                sq = squares_stage(rc, r0, r1)
                if sq_pend is not None:
                    t1t2v = rot_stage(*sq_pend)
                    rot_done.append((sq_pend[0], sq_pend[1], sq_pend[2], *t1t2v))
                    if len(mm_done) > 1:
                        mm_evict(*mm_done.pop(0))
                    if len(rot_done) > 1:
                        args = rot_done.pop(0)
                        psts = mm_matmuls(*args)
                        mm_done.append((args[0], args[1], args[2], psts))
                sq_pend = (rc, r0, r1, *sq)
            t1t2v = rot_stage(*sq_pend)
            rot_done.append((sq_pend[0], sq_pend[1], sq_pend[2], *t1t2v))
            for args in rot_done:
                psts = mm_matmuls(*args)
                mm_done.append((args[0], args[1], args[2], psts))
                while len(mm_done) > 1:
                    mm_evict(*mm_done.pop(0))
            while mm_done:
                mm_evict(*mm_done.pop(0))
    return nc


def kernel(psi_r, psi_i, alpha, ham_w):
    psi_r = np.asarray(psi_r, dtype=np.float32)
    psi_i = np.asarray(psi_i, dtype=np.float32)
    alpha = np.asarray(alpha, dtype=np.float32)

    uniform = bool(np.all(alpha == alpha.flat[0]))
    h = _pick_h(ham_w)
    key = ("prog", h, uniform)
    if key not in _cache:
        nc = _build_program(h, uniform)
        nc.finalize()
        _cache[key] = nc
    nc = _cache[key]
    _cache[("nc", uniform)] = nc  # test.py compatibility

    mband = _host_mband(ham_w, h)

    # host-side normalisation fold: k_row = alpha_scale / (mean I + 1e-8)
    pr = psi_r.reshape(B * S, D)
    pi = psi_i.reshape(B * S, D)
    inten_mean = (
        (pr.astype(np.float64) ** 2 + pi.astype(np.float64) ** 2).mean(axis=1)
    )
    k_row = 1.0 / (inten_mean + 1e-8)
    k_glob = float(np.exp(np.mean(np.log(k_row))))
    s_row = np.sqrt(k_row / k_glob)          # pre-scale; exp(log-mean) keeps ~1
    # per-d activation scale alpha[d] * k_glob, laid out [p, dc] (d = dc*128+p)
    kg = np.ascontiguousarray(
        (alpha * k_glob).reshape(N_DC, 128).T.astype(np.float32)
    )

    sc = s_row.astype(np.float32)[:, None]
    prT = np.ascontiguousarray((pr * sc).T.astype(np.float16))
    piT = np.ascontiguousarray((pi * sc).T.astype(np.float16))

    in_maps = []
    for c in range(N_CORES):
        sl = slice(c * ROWS, (c + 1) * ROWS)
        in_maps.append(
            {
                "psi_rt": np.ascontiguousarray(prT[:, sl]),
                "psi_it": np.ascontiguousarray(piT[:, sl]),
                "mband": mband,
                "kg": kg,
            }
        )
    res = run_bass_kernel_spmd(nc, in_maps, core_ids=list(range(N_CORES)))
    _cache["last_run"] = res
    out16 = np.concatenate([r["out"] for r in res.results], axis=0)
    # [rows, 2, D] fp16 -> [rows, D, 2] f32, descale rows by 1/s_row
    full = out16.reshape(B * S, 2, D).astype(np.float32)
    full *= (1.0 / s_row).astype(np.float32)[:, None, None]
    return np.ascontiguousarray(full.transpose(0, 2, 1)).reshape(B, S, D, 2)
